# revision 35
# baseline (speedup 1.0000x reference)
"""MHA kernel for TRN2: B=4,T=2048,D=1024,H=16,HD=64 across 8 NeuronCores.

Sharding: core c -> batch c//2, query half c%2 (host rotates the sequence so
each core's queries are columns 0:1024; softmax over keys is permutation
invariant). No collectives.

v5: all-bf16 matmul operands (rel err ~3e-3), host-side pre-transpose + cast
(no PE transposes), software-pipelined emission: K/Q projections for the next
head group, V-projection chunks, and the final projection are split into
single-matmul steps and drained from work queues inside the attention loop, so
the in-order PE stream always has independent work while PV waits on ACT exp.
V chunks for group 0 / group 7 are emitted in sc-lockstep inside that group's
own attention (3-chunk lookahead). Softmax denominators come from a ones
column in V ([65,512] PV psum row 64), replicated across partitions with a
0-stride-AP SBUF DMA broadcast, then DVE reciprocal+multiply.
"""
import sys
sys.path.insert(0, "/opt/trn_rl_repo")
import warnings
warnings.filterwarnings("ignore")

from collections import deque

import numpy as np
import ml_dtypes
import concourse.bass as bass
import concourse.mybir as mybir
import concourse.tile as tile
from concourse import bacc
from concourse.bass_utils import run_bass_kernel_spmd

F32 = mybir.dt.float32
BF16 = mybir.dt.bfloat16
EXP = mybir.ActivationFunctionType.Exp

T, D = 2048, 1024
TQ = 1024          # queries per core
NG = 8             # head groups (2 heads each)
NSC = 16           # s chunks of 128
NDC = 8            # d chunks of 128
SCALE = 0.125      # 1/sqrt(64)


def build_nc():
    nc = bacc.Bacc("TRN2", target_bir_lowering=False, debug=False, num_devices=8)
    # host-prepped layouts (partition-major, bf16)
    xin = nc.dram_tensor("xin", [128, NDC, T], BF16, kind="ExternalInput")
    wqt = nc.dram_tensor("wqt", [128, NDC, NG, 128], BF16, kind="ExternalInput")
    wkt = nc.dram_tensor("wkt", [128, NDC, NG, 128], BF16, kind="ExternalInput")
    wvt = nc.dram_tensor("wvt", [128, NDC, NG, 128], BF16, kind="ExternalInput")
    wot = nc.dram_tensor("wot", [128, NG, D], BF16, kind="ExternalInput")
    bo = nc.dram_tensor("bo", [1, D], F32, kind="ExternalInput")
    y = nc.dram_tensor("y", [TQ, D], F32, kind="ExternalOutput")

    with tile.TileContext(nc) as tc:
        with (
            tc.tile_pool(name="persist", bufs=1) as pp,
            tc.tile_pool(name="kq", bufs=2) as kqp,
            tc.tile_pool(name="ptp", bufs=4) as ptp,
            tc.tile_pool(name="small", bufs=3) as sp,
            tc.tile_pool(name="yt", bufs=3) as yp,
            tc.tile_pool(name="ps_work", bufs=2, space="PSUM") as psw,
            tc.tile_pool(name="ps_log", bufs=2, space="PSUM") as psl,
            tc.tile_pool(name="ps_pv", bufs=2, space="PSUM") as psv,
        ):
            # input DMAs: interleave weight and x chunks so the in-order PE
            # stream unblocks as early as possible
            wkT = pp.tile([128, NDC, NG, 128], BF16, name="wkT")
            wqT = pp.tile([128, NDC, NG, 128], BF16, name="wqT")
            wvT = pp.tile([128, NDC, NG, 128], BF16, name="wvT")
            xT = pp.tile([128, NDC, T], BF16, name="xT")
            for wT, wsrc, q4 in ((wkT, wkt, 0), (wqT, wqt, 1), (wvT, wvt, 2)):
                qsl = slice(q4 * 512, (q4 + 1) * 512)
                for dc in range(NDC):
                    nc.sync.dma_start(out=wT[:, dc], in_=wsrc[:, dc])
                    nc.sync.dma_start(out=xT[:, dc, qsl], in_=xin[:, dc, qsl])
            for dc in range(NDC):
                nc.sync.dma_start(out=xT[:, dc, 1536:2048], in_=xin[:, dc, 1536:2048])
            woT = pp.tile([128, NG, D], BF16, name="woT")
            nc.sync.dma_start(out=woT, in_=wot[:, :, :])
            bias = pp.tile([128, D], F32)
            nc.sync.dma_start(
                out=bias, in_=bass.AP(tensor=bo, offset=0, ap=[[0, 128], [1, D]]))

            catT = pp.tile([128, NG, TQ], BF16, name="catT")
            vt = pp.tile([128, NSC, 16, 65], BF16, name="vt")
            nc.gpsimd.memset(vt[:, :, :, 64:65], 1.0)

            high = deque()   # K/Q proj for the next head group, final proj
            low = deque()    # V-projection chunks (deadline several groups out)
            vlock = {}       # g -> deque of whole V chunks, sc-lockstep drained

            def drain(n):
                for _ in range(n):
                    if high:
                        high.popleft()()
                    elif low:
                        low.popleft()()
                    else:
                        break

            def v_chunk_steps(sc, g0, ng):
                """V proj steps for head groups g0..g0+ng at s-chunk sc."""
                w = ng * 128
                st = {}
                def mm(dc):
                    if dc == 0:
                        st["p"] = psw.tile([128, 512], F32, tag="work",
                                           name=f"vw{g0}_{sc}")
                    nc.tensor.matmul(
                        st["p"][:, 0:w],
                        xT[:, dc, sc * 128:(sc + 1) * 128],
                        wvT[:, dc, g0:g0 + ng, :],
                        start=(dc == 0), stop=(dc == NDC - 1))
                def cp():
                    nc.vector.tensor_copy(
                        out=vt[:, sc, 2 * g0:2 * (g0 + ng), 0:64],
                        in_=st["p"][:, 0:w].rearrange("p (h c) -> p h c", h=2 * ng))
                return [lambda dc=dc: mm(dc) for dc in range(NDC)] + [cp]

            def push_v_low(g0, ng):
                for sc in range(NSC):
                    low.extend(v_chunk_steps(sc, g0, ng))

            def push_v_lock(g, g0, ng):
                dq = deque()
                for sc in range(NSC):
                    def chunk(sc=sc, g0=g0, ng=ng):
                        for f in v_chunk_steps(sc, g0, ng):
                            f()
                    dq.append(chunk)
                vlock[g] = dq

            kqtiles = {}

            def push_proj(g, sbs=(0, 1, 2, 3), qhs=(0, 1)):
                # K/Q projections for head group g -> high queue
                if g in kqtiles:
                    kT, qT = kqtiles[g]
                else:
                    kT = kqp.tile([128, T], BF16, tag="kT")
                    qT = kqp.tile([128, TQ], BF16, tag="qT")
                    kqtiles[g] = (kT, qT)
                for sb in sbs:
                    st = {}
                    def mk(sb=sb, st=st, kT=kT):
                        def mm(dc):
                            if dc == 0:
                                st["p"] = psw.tile([128, 512], F32, tag="work",
                                                   name=f"kw{g}_{sb}")
                            nc.tensor.matmul(
                                st["p"], wkT[:, dc, g, :],
                                xT[:, dc, sb * 512:(sb + 1) * 512],
                                start=(dc == 0), stop=(dc == NDC - 1))
                        def cp():
                            nc.vector.tensor_copy(
                                out=kT[:, sb * 512:(sb + 1) * 512], in_=st["p"])
                        return [lambda dc=dc: mm(dc) for dc in range(NDC)] + [cp]
                    high.extend(mk())
                for qh in qhs:
                    st = {}
                    def mk(qh=qh, st=st, qT=qT):
                        qs = slice(qh * 512, (qh + 1) * 512)
                        def mm(dc):
                            if dc == 0:
                                st["p"] = psw.tile([128, 512], F32, tag="work",
                                                   name=f"qw{g}_{qh}")
                            nc.tensor.matmul(
                                st["p"], wqT[:, dc, g, :],
                                xT[:, dc, qh * 512:(qh + 1) * 512],
                                start=(dc == 0), stop=(dc == NDC - 1))
                        def cp():
                            nc.vector.tensor_copy(out=qT[:, qs], in_=st["p"])
                        return [lambda dc=dc: mm(dc) for dc in range(NDC)] + [cp]
                    high.extend(mk())

            def push_final(qbs):
                # final projection y[qb*128:, :] = catT.T @ woT + bias
                for qb in qbs:
                    yt = yp.tile([128, D], F32, tag="yt", name=f"yt{qb}")
                    for nh in range(2):
                        st = {}
                        def mk(qb=qb, nh=nh, st=st, yt=yt):
                            def mm(g2):
                                if g2 == 0:
                                    st["p"] = psw.tile([128, 512], F32, tag="work",
                                                       name=f"fw{qb}_{nh}")
                                nc.tensor.matmul(
                                    st["p"],
                                    catT[:, g2, qb * 128:(qb + 1) * 128],
                                    woT[:, g2, nh * 512:(nh + 1) * 512],
                                    start=(g2 == 0), stop=(g2 == NG - 1))
                            def add():
                                nc.vector.tensor_add(
                                    out=yt[:, nh * 512:(nh + 1) * 512],
                                    in0=st["p"],
                                    in1=bias[:, nh * 512:(nh + 1) * 512])
                            return ([lambda g2=g2: mm(g2) for g2 in range(NG)]
                                    + [add])
                        high.extend(mk())
                    def out_dma(qb=qb, yt=yt):
                        nc.sync.dma_start(
                            out=y[qb * 128:(qb + 1) * 128, :], in_=yt)
                    high.append(out_dma)

            # V schedule: groups 0,1 lockstep in g0; 2..6 via low queue;
            # group 7 lockstep in g7.
            push_v_lock(0, 0, 2)
            push_proj(0, sbs=(0,), qhs=(0,))
            while high:
                drain(1)
            push_proj(0, sbs=(1, 2, 3), qhs=(1,))
            push_v_low(2, 2)
            push_v_low(4, 2)
            push_v_low(6, 1)
            push_v_lock(7, 7, 1)
            # max low steps allowed to remain at the start of group g
            allowed = {0: 432, 1: 432, 2: 288, 3: 288, 4: 144, 5: 144, 6: 0, 7: 0}

            for g in range(NG):
                if g + 1 < NG:
                    push_proj(g + 1)
                while len(low) > allowed[g]:
                    low.popleft()()
                kT, qT = kqtiles[g]
                vl = vlock.get(g)
                emitted = 0
                for qh in range(2):
                    qs = slice(qh * 512, (qh + 1) * 512)
                    pv0 = psv.tile([65, 512], F32, tag="pv")
                    pv1 = psv.tile([65, 512], F32, tag="pv")
                    for sc in range(NSC):
                        if vl and qh == 0:
                            while vl and emitted < min(sc + 2, NSC):
                                vl.popleft()()
                                emitted += 1
                        lg = psl.tile([128, 2, 512], F32, tag="log")
                        nc.tensor.matmul(
                            lg[:, 0, :], kT[0:64, sc * 128:(sc + 1) * 128],
                            qT[0:64, qs], start=True, stop=True)
                        nc.tensor.matmul(
                            lg[:, 1, :], kT[64:128, sc * 128:(sc + 1) * 128],
                            qT[64:128, qs], start=True, stop=True)
                        pt = ptp.tile([128, 2, 512], BF16, tag="pt")
                        nc.scalar.activation(
                            out=pt.rearrange("p a b -> p (a b)"),
                            in_=lg.rearrange("p a b -> p (a b)"),
                            func=EXP, scale=SCALE)
                        drain(1)
                        nc.tensor.matmul(
                            pv0, vt[:, sc, 2 * g, :], pt[:, 0, :],
                            start=(sc == 0), stop=(sc == NSC - 1))
                        nc.tensor.matmul(
                            pv1, vt[:, sc, 2 * g + 1, :], pt[:, 1, :],
                            start=(sc == 0), stop=(sc == NSC - 1))
                        drain(3)
                    for hloc, pv in ((0, pv0), (1, pv1)):
                        # denominator: reciprocal of PV row 64 on DVE, then
                        # replicate across partitions with a 0-stride DMA
                        rrow = sp.tile([65, 512], F32, tag="rrow")
                        nc.vector.reciprocal(out=rrow[64:65, :], in_=pv[64:65, :])
                        rec = sp.tile([64, 512], F32, tag="rec")
                        row = rrow[64:65, :]
                        brd = bass.AP(
                            tensor=row.tensor, offset=row.offset,
                            ap=[list(row.ap[0]), [0, 64]]
                               + [list(d) for d in row.ap[1:]])
                        nc.sync.dma_start(out=rec, in_=brd)
                        if hloc == 0:
                            nc.vector.tensor_mul(
                                out=catT[0:64, g, qs], in0=pv[0:64, :], in1=rec)
                        else:
                            tmp = sp.tile([64, 512], BF16, tag="tmp")
                            nc.vector.tensor_mul(out=tmp, in0=pv[0:64, :], in1=rec)
                            nc.sync.dma_start(out=catT[64:128, g, qs], in_=tmp)
                    if g == NG - 1:
                        # final projection for the completed query half
                        push_final([0, 1, 2, 3] if qh == 0 else [4, 5, 6, 7])

            while high or low:
                drain(1)

    nc.compile()
    return nc


_CACHE = {}


def _prep_weights(Wq, Wk, Wv, Wo, bo):
    def wt(W):  # [1024 out, 1024 in] -> [128 p, dc, g, 128 j] bf16
        WT = np.ascontiguousarray(W.reshape(D, D)).T  # [in, out]
        return np.ascontiguousarray(
            WT.reshape(NDC, 128, NG, 128).transpose(1, 0, 2, 3)
        ).astype(ml_dtypes.bfloat16)
    wot = np.ascontiguousarray(
        Wo.T.reshape(NG, 128, D).transpose(1, 0, 2)).astype(ml_dtypes.bfloat16)
    bo2 = np.ascontiguousarray(bo.reshape(1, D)).astype(np.float32)
    return wt(Wq), wt(Wk), wt(Wv), wot, bo2


def kernel(x, Wq, Wk, Wv, Wo, bo):
    if "nc" not in _CACHE:
        _CACHE["nc"] = build_nc()
    nc = _CACHE["nc"]
    wq2, wk2, wv2, wot, bo2 = _prep_weights(
        np.asarray(Wq, np.float32), np.asarray(Wk, np.float32),
        np.asarray(Wv, np.float32), np.asarray(Wo, np.float32),
        np.asarray(bo, np.float32))
    x = np.asarray(x, np.float32)
    in_maps = []
    for c in range(8):
        b, h = c // 2, c % 2
        xb = x[b] if h == 0 else np.concatenate([x[b, TQ:], x[b, :TQ]], axis=0)
        # x^T [1024 d, 2048 s] -> [128 p, dc, s] bf16
        xT = np.ascontiguousarray(
            xb.T.reshape(NDC, 128, T).transpose(1, 0, 2)).astype(ml_dtypes.bfloat16)
        in_maps.append({"xin": xT, "wqt": wq2, "wkt": wk2, "wvt": wv2,
                        "wot": wot, "bo": bo2})
    res = run_bass_kernel_spmd(nc, in_maps, core_ids=list(range(8)))
    out = np.empty((4, T, D), dtype=np.float32)
    for c in range(8):
        b, h = c // 2, c % 2
        out[b, h * TQ:(h + 1) * TQ] = res.results[c]["y"]
    return out


# revision 36
# speedup vs baseline: 1.0142x; 1.0142x over previous
"""MHA kernel for TRN2: B=4,T=2048,D=1024,H=16,HD=64 across 8 NeuronCores.

Sharding: core c -> batch c//2, query half c%2 (host rotates the sequence so
each core's queries are columns 0:1024; softmax over keys is permutation
invariant). No collectives.

v5: all-bf16 matmul operands (rel err ~3e-3), host-side pre-transpose + cast
(no PE transposes), software-pipelined emission: K/Q projections for the next
head group, V-projection chunks, and the final projection are split into
single-matmul steps and drained from work queues inside the attention loop, so
the in-order PE stream always has independent work while PV waits on ACT exp.
V chunks for group 0 / group 7 are emitted in sc-lockstep inside that group's
own attention (3-chunk lookahead). Softmax denominators come from a ones
column in V ([65,512] PV psum row 64), replicated across partitions with a
0-stride-AP SBUF DMA broadcast, then DVE reciprocal+multiply.
"""
import sys
sys.path.insert(0, "/opt/trn_rl_repo")
import warnings
warnings.filterwarnings("ignore")

from collections import deque

import numpy as np
import ml_dtypes
import concourse.bass as bass
import concourse.mybir as mybir
import concourse.tile as tile
from concourse import bacc
from concourse.bass_utils import run_bass_kernel_spmd

F32 = mybir.dt.float32
BF16 = mybir.dt.bfloat16
EXP = mybir.ActivationFunctionType.Exp

T, D = 2048, 1024
TQ = 1024          # queries per core
NG = 8             # head groups (2 heads each)
NSC = 16           # s chunks of 128
NDC = 8            # d chunks of 128
SCALE = 0.125      # 1/sqrt(64)


def build_nc():
    nc = bacc.Bacc("TRN2", target_bir_lowering=False, debug=False, num_devices=8)
    # host-prepped layouts (partition-major, bf16)
    xin = nc.dram_tensor("xin", [128, NDC, T], BF16, kind="ExternalInput")
    wqt = nc.dram_tensor("wqt", [128, NDC, NG, 128], BF16, kind="ExternalInput")
    wkt = nc.dram_tensor("wkt", [128, NDC, NG, 128], BF16, kind="ExternalInput")
    wvt = nc.dram_tensor("wvt", [128, NDC, NG, 128], BF16, kind="ExternalInput")
    wot = nc.dram_tensor("wot", [128, NG, D], BF16, kind="ExternalInput")
    bo = nc.dram_tensor("bo", [1, D], F32, kind="ExternalInput")
    y = nc.dram_tensor("y", [TQ, D], F32, kind="ExternalOutput")

    with tile.TileContext(nc) as tc:
        with (
            tc.tile_pool(name="persist", bufs=1) as pp,
            tc.tile_pool(name="kq", bufs=2) as kqp,
            tc.tile_pool(name="ptp", bufs=4) as ptp,
            tc.tile_pool(name="small", bufs=3) as sp,
            tc.tile_pool(name="yt", bufs=3) as yp,
            tc.tile_pool(name="ps_work", bufs=2, space="PSUM") as psw,
            tc.tile_pool(name="ps_log", bufs=2, space="PSUM") as psl,
            tc.tile_pool(name="ps_pv", bufs=2, space="PSUM") as psv,
        ):
            # input DMAs: interleave weight and x chunks so the in-order PE
            # stream unblocks as early as possible
            wkT = pp.tile([128, NDC, NG, 128], BF16, name="wkT")
            wqT = pp.tile([128, NDC, NG, 128], BF16, name="wqT")
            wvT = pp.tile([128, NDC, NG, 128], BF16, name="wvT")
            xT = pp.tile([128, NDC, T], BF16, name="xT")
            for dc in range(NDC):
                nc.sync.dma_start(out=wkT[:, dc], in_=wkt[:, dc])
                nc.sync.dma_start(out=wqT[:, dc], in_=wqt[:, dc])
                nc.sync.dma_start(out=wvT[:, dc], in_=wvt[:, dc])
                nc.sync.dma_start(out=xT[:, dc, 0:512], in_=xin[:, dc, 0:512])
            for q4 in range(1, 4):
                qsl = slice(q4 * 512, (q4 + 1) * 512)
                for dc in range(NDC):
                    nc.sync.dma_start(out=xT[:, dc, qsl], in_=xin[:, dc, qsl])
            woT = pp.tile([128, NG, D], BF16, name="woT")
            nc.sync.dma_start(out=woT, in_=wot[:, :, :])
            bias = pp.tile([128, D], F32)
            nc.sync.dma_start(
                out=bias, in_=bass.AP(tensor=bo, offset=0, ap=[[0, 128], [1, D]]))

            catT = pp.tile([128, NG, TQ], BF16, name="catT")
            vt = pp.tile([128, NSC, 16, 65], BF16, name="vt")
            nc.gpsimd.memset(vt[:, :, :, 64:65], 1.0)

            high = deque()   # K/Q proj for the next head group, final proj
            low = deque()    # V-projection chunks (deadline several groups out)
            vlock = {}       # g -> deque of whole V chunks, sc-lockstep drained

            def drain(n):
                for _ in range(n):
                    if high:
                        high.popleft()()
                    elif low:
                        low.popleft()()
                    else:
                        break

            def v_chunk_steps(sc, g0, ng):
                """V proj steps for head groups g0..g0+ng at s-chunk sc."""
                w = ng * 128
                st = {}
                def mm(dc):
                    if dc == 0:
                        st["p"] = psw.tile([128, 512], F32, tag="work",
                                           name=f"vw{g0}_{sc}")
                    nc.tensor.matmul(
                        st["p"][:, 0:w],
                        xT[:, dc, sc * 128:(sc + 1) * 128],
                        wvT[:, dc, g0:g0 + ng, :],
                        start=(dc == 0), stop=(dc == NDC - 1))
                def cp():
                    nc.vector.tensor_copy(
                        out=vt[:, sc, 2 * g0:2 * (g0 + ng), 0:64],
                        in_=st["p"][:, 0:w].rearrange("p (h c) -> p h c", h=2 * ng))
                return [lambda dc=dc: mm(dc) for dc in range(NDC)] + [cp]

            def push_v_low(g0, ng):
                for sc in range(NSC):
                    low.extend(v_chunk_steps(sc, g0, ng))

            def push_v_lock(g, g0, ng):
                dq = deque()
                for sc in range(NSC):
                    def chunk(sc=sc, g0=g0, ng=ng):
                        for f in v_chunk_steps(sc, g0, ng):
                            f()
                    dq.append(chunk)
                vlock[g] = dq

            kqtiles = {}

            def push_proj(g, sbs=(0, 1, 2, 3), qhs=(0, 1)):
                # K/Q projections for head group g -> high queue
                if g in kqtiles:
                    kT, qT = kqtiles[g]
                else:
                    kT = kqp.tile([128, T], BF16, tag="kT")
                    qT = kqp.tile([128, TQ], BF16, tag="qT")
                    kqtiles[g] = (kT, qT)
                for sb in sbs:
                    st = {}
                    def mk(sb=sb, st=st, kT=kT):
                        def mm(dc):
                            if dc == 0:
                                st["p"] = psw.tile([128, 512], F32, tag="work",
                                                   name=f"kw{g}_{sb}")
                            nc.tensor.matmul(
                                st["p"], wkT[:, dc, g, :],
                                xT[:, dc, sb * 512:(sb + 1) * 512],
                                start=(dc == 0), stop=(dc == NDC - 1))
                        def cp():
                            nc.vector.tensor_copy(
                                out=kT[:, sb * 512:(sb + 1) * 512], in_=st["p"])
                        return [lambda dc=dc: mm(dc) for dc in range(NDC)] + [cp]
                    high.extend(mk())
                for qh in qhs:
                    st = {}
                    def mk(qh=qh, st=st, qT=qT):
                        qs = slice(qh * 512, (qh + 1) * 512)
                        def mm(dc):
                            if dc == 0:
                                st["p"] = psw.tile([128, 512], F32, tag="work",
                                                   name=f"qw{g}_{qh}")
                            nc.tensor.matmul(
                                st["p"], wqT[:, dc, g, :],
                                xT[:, dc, qh * 512:(qh + 1) * 512],
                                start=(dc == 0), stop=(dc == NDC - 1))
                        def cp():
                            nc.vector.tensor_copy(out=qT[:, qs], in_=st["p"])
                        return [lambda dc=dc: mm(dc) for dc in range(NDC)] + [cp]
                    high.extend(mk())

            def push_final(qbs):
                # final projection y[qb*128:, :] = catT.T @ woT + bias
                for qb in qbs:
                    yt = yp.tile([128, D], F32, tag="yt", name=f"yt{qb}")
                    for nh in range(2):
                        st = {}
                        def mk(qb=qb, nh=nh, st=st, yt=yt):
                            def mm(g2):
                                if g2 == 0:
                                    st["p"] = psw.tile([128, 512], F32, tag="work",
                                                       name=f"fw{qb}_{nh}")
                                nc.tensor.matmul(
                                    st["p"],
                                    catT[:, g2, qb * 128:(qb + 1) * 128],
                                    woT[:, g2, nh * 512:(nh + 1) * 512],
                                    start=(g2 == 0), stop=(g2 == NG - 1))
                            def add():
                                nc.vector.tensor_add(
                                    out=yt[:, nh * 512:(nh + 1) * 512],
                                    in0=st["p"],
                                    in1=bias[:, nh * 512:(nh + 1) * 512])
                            return ([lambda g2=g2: mm(g2) for g2 in range(NG)]
                                    + [add])
                        high.extend(mk())
                    def out_dma(qb=qb, yt=yt):
                        nc.sync.dma_start(
                            out=y[qb * 128:(qb + 1) * 128, :], in_=yt)
                    high.append(out_dma)

            # V schedule: groups 0,1 lockstep in g0; 2..6 via low queue;
            # group 7 lockstep in g7.
            push_v_lock(0, 0, 2)
            push_proj(0, sbs=(0,), qhs=(0,))
            while high:
                drain(1)
            push_proj(0, sbs=(1, 2, 3), qhs=(1,))
            push_v_low(2, 2)
            push_v_low(4, 2)
            push_v_low(6, 1)
            push_v_lock(7, 7, 1)
            # max low steps allowed to remain at the start of group g
            allowed = {0: 432, 1: 432, 2: 288, 3: 288, 4: 144, 5: 144, 6: 0, 7: 0}

            for g in range(NG):
                if g + 1 < NG:
                    push_proj(g + 1)
                while len(low) > allowed[g]:
                    low.popleft()()
                kT, qT = kqtiles[g]
                vl = vlock.get(g)
                emitted = 0
                for qh in range(2):
                    qs = slice(qh * 512, (qh + 1) * 512)
                    pv0 = psv.tile([65, 512], F32, tag="pv")
                    pv1 = psv.tile([65, 512], F32, tag="pv")
                    for sc in range(NSC):
                        if vl and qh == 0:
                            while vl and emitted < min(sc + 2, NSC):
                                vl.popleft()()
                                emitted += 1
                        lg = psl.tile([128, 2, 512], F32, tag="log")
                        nc.tensor.matmul(
                            lg[:, 0, :], kT[0:64, sc * 128:(sc + 1) * 128],
                            qT[0:64, qs], start=True, stop=True)
                        nc.tensor.matmul(
                            lg[:, 1, :], kT[64:128, sc * 128:(sc + 1) * 128],
                            qT[64:128, qs], start=True, stop=True)
                        pt = ptp.tile([128, 2, 512], BF16, tag="pt")
                        nc.scalar.activation(
                            out=pt.rearrange("p a b -> p (a b)"),
                            in_=lg.rearrange("p a b -> p (a b)"),
                            func=EXP, scale=SCALE)
                        drain(1)
                        nc.tensor.matmul(
                            pv0, vt[:, sc, 2 * g, :], pt[:, 0, :],
                            start=(sc == 0), stop=(sc == NSC - 1))
                        nc.tensor.matmul(
                            pv1, vt[:, sc, 2 * g + 1, :], pt[:, 1, :],
                            start=(sc == 0), stop=(sc == NSC - 1))
                        drain(3)
                    for hloc, pv in ((0, pv0), (1, pv1)):
                        # denominator: reciprocal of PV row 64 on DVE, then
                        # replicate across partitions with a 0-stride DMA
                        rrow = sp.tile([65, 512], F32, tag="rrow")
                        nc.vector.reciprocal(out=rrow[64:65, :], in_=pv[64:65, :])
                        rec = sp.tile([64, 512], F32, tag="rec")
                        row = rrow[64:65, :]
                        brd = bass.AP(
                            tensor=row.tensor, offset=row.offset,
                            ap=[list(row.ap[0]), [0, 64]]
                               + [list(d) for d in row.ap[1:]])
                        nc.sync.dma_start(out=rec, in_=brd)
                        if hloc == 0:
                            nc.vector.tensor_mul(
                                out=catT[0:64, g, qs], in0=pv[0:64, :], in1=rec)
                        else:
                            tmp = sp.tile([64, 512], BF16, tag="tmp")
                            nc.vector.tensor_mul(out=tmp, in0=pv[0:64, :], in1=rec)
                            nc.sync.dma_start(out=catT[64:128, g, qs], in_=tmp)
                    if g == NG - 1:
                        # final projection for the completed query half
                        push_final([0, 1, 2, 3] if qh == 0 else [4, 5, 6, 7])

            while high or low:
                drain(1)

    nc.compile()
    return nc


_CACHE = {}


def _prep_weights(Wq, Wk, Wv, Wo, bo):
    def wt(W):  # [1024 out, 1024 in] -> [128 p, dc, g, 128 j] bf16
        WT = np.ascontiguousarray(W.reshape(D, D)).T  # [in, out]
        return np.ascontiguousarray(
            WT.reshape(NDC, 128, NG, 128).transpose(1, 0, 2, 3)
        ).astype(ml_dtypes.bfloat16)
    wot = np.ascontiguousarray(
        Wo.T.reshape(NG, 128, D).transpose(1, 0, 2)).astype(ml_dtypes.bfloat16)
    bo2 = np.ascontiguousarray(bo.reshape(1, D)).astype(np.float32)
    return wt(Wq), wt(Wk), wt(Wv), wot, bo2


def kernel(x, Wq, Wk, Wv, Wo, bo):
    if "nc" not in _CACHE:
        _CACHE["nc"] = build_nc()
    nc = _CACHE["nc"]
    wq2, wk2, wv2, wot, bo2 = _prep_weights(
        np.asarray(Wq, np.float32), np.asarray(Wk, np.float32),
        np.asarray(Wv, np.float32), np.asarray(Wo, np.float32),
        np.asarray(bo, np.float32))
    x = np.asarray(x, np.float32)
    in_maps = []
    for c in range(8):
        b, h = c // 2, c % 2
        xb = x[b] if h == 0 else np.concatenate([x[b, TQ:], x[b, :TQ]], axis=0)
        # x^T [1024 d, 2048 s] -> [128 p, dc, s] bf16
        xT = np.ascontiguousarray(
            xb.T.reshape(NDC, 128, T).transpose(1, 0, 2)).astype(ml_dtypes.bfloat16)
        in_maps.append({"xin": xT, "wqt": wq2, "wkt": wk2, "wvt": wv2,
                        "wot": wot, "bo": bo2})
    res = run_bass_kernel_spmd(nc, in_maps, core_ids=list(range(8)))
    out = np.empty((4, T, D), dtype=np.float32)
    for c in range(8):
        b, h = c // 2, c % 2
        out[b, h * TQ:(h + 1) * TQ] = res.results[c]["y"]
    return out


# revision 43
# speedup vs baseline: 1.0321x; 1.0177x over previous
"""MHA kernel for TRN2: B=4,T=2048,D=1024,H=16,HD=64 across 8 NeuronCores.

Sharding: core c -> batch c//2, query half c%2 (host rotates the sequence so
each core's queries are columns 0:1024; softmax over keys is permutation
invariant). No collectives.

v5: all-bf16 matmul operands (rel err ~3e-3), host-side pre-transpose + cast
(no PE transposes), software-pipelined emission: K/Q projections for the next
head group, V-projection chunks, and the final projection are split into
single-matmul steps and drained from work queues inside the attention loop, so
the in-order PE stream always has independent work while PV waits on ACT exp.
V chunks for group 0 / group 7 are emitted in sc-lockstep inside that group's
own attention (3-chunk lookahead). Softmax denominators come from a ones
column in V ([65,512] PV psum row 64), replicated across partitions with a
0-stride-AP SBUF DMA broadcast, then DVE reciprocal+multiply.
"""
import sys
sys.path.insert(0, "/opt/trn_rl_repo")
import warnings
warnings.filterwarnings("ignore")

from collections import deque

import numpy as np
import ml_dtypes
import concourse.bass as bass
import concourse.mybir as mybir
import concourse.tile as tile
from concourse import bacc
from concourse.bass_utils import run_bass_kernel_spmd

F32 = mybir.dt.float32
BF16 = mybir.dt.bfloat16
EXP = mybir.ActivationFunctionType.Exp

T, D = 2048, 1024
TQ = 1024          # queries per core
NG = 8             # head groups (2 heads each)
NSC = 16           # s chunks of 128
NDC = 8            # d chunks of 128
SCALE = 0.125      # 1/sqrt(64)


def build_nc():
    nc = bacc.Bacc("TRN2", target_bir_lowering=False, debug=False, num_devices=8)
    # host-prepped layouts (partition-major, bf16)
    xin = nc.dram_tensor("xin", [128, NDC, T], BF16, kind="ExternalInput")
    wqt = nc.dram_tensor("wqt", [128, NDC, NG, 128], BF16, kind="ExternalInput")
    wkt = nc.dram_tensor("wkt", [128, NDC, NG, 128], BF16, kind="ExternalInput")
    wvt = nc.dram_tensor("wvt", [128, NDC, NG, 128], BF16, kind="ExternalInput")
    wot = nc.dram_tensor("wot", [128, NG, D], BF16, kind="ExternalInput")
    wk01 = nc.dram_tensor("wk01", [128, NDC, 2, 128], BF16, kind="ExternalInput")
    wq01 = nc.dram_tensor("wq01", [128, NDC, 2, 128], BF16, kind="ExternalInput")
    wv01 = nc.dram_tensor("wv01", [128, NDC, 2, 128], BF16, kind="ExternalInput")
    bo = nc.dram_tensor("bo", [1, D], F32, kind="ExternalInput")
    y = nc.dram_tensor("y", [TQ, D], F32, kind="ExternalOutput")

    with tile.TileContext(nc) as tc:
        with (
            tc.tile_pool(name="persist", bufs=1) as pp,
            tc.tile_pool(name="kq", bufs=2) as kqp,
            tc.tile_pool(name="ptp", bufs=5) as ptp,
            tc.tile_pool(name="small", bufs=4) as sp,
            tc.tile_pool(name="yt", bufs=3) as yp,
            tc.tile_pool(name="ps_work", bufs=2, space="PSUM") as psw,
            tc.tile_pool(name="ps_log", bufs=2, space="PSUM") as psl,
            tc.tile_pool(name="ps_pv", bufs=2, space="PSUM") as psv,
        ):
            # input DMAs: interleave weight and x chunks so the in-order PE
            # stream unblocks as early as possible
            wkT = pp.tile([128, NDC, NG, 128], BF16, name="wkT")
            wqT = pp.tile([128, NDC, NG, 128], BF16, name="wqT")
            wvT = pp.tile([128, NDC, NG, 128], BF16, name="wvT")
            xT = pp.tile([128, NDC, T], BF16, name="xT")
            nc.sync.dma_start(out=wkT[:, :, 0:2, :], in_=wk01[:, :, :, :])
            nc.sync.dma_start(out=wqT[:, :, 0:2, :], in_=wq01[:, :, :, :])
            nc.sync.dma_start(out=wvT[:, :, 0:2, :], in_=wv01[:, :, :, :])
            for dc in range(NDC):
                nc.sync.dma_start(out=xT[:, dc, 0:512], in_=xin[:, dc, 0:512])
            for dc in range(NDC):
                nc.sync.dma_start(out=xT[:, dc, 512:1024], in_=xin[:, dc, 512:1024])
            nc.sync.dma_start(out=wkT[:, :, 2:8, :], in_=wkt[:, :, 2:8, :])
            nc.sync.dma_start(out=wqT[:, :, 2:8, :], in_=wqt[:, :, 2:8, :])
            for dc in range(NDC):
                nc.sync.dma_start(out=xT[:, dc, 1024:1536], in_=xin[:, dc, 1024:1536])
            nc.sync.dma_start(out=wvT[:, :, 2:8, :], in_=wvt[:, :, 2:8, :])
            for dc in range(NDC):
                nc.sync.dma_start(out=xT[:, dc, 1536:2048], in_=xin[:, dc, 1536:2048])
            woT = pp.tile([128, NG, D], BF16, name="woT")
            nc.sync.dma_start(out=woT, in_=wot[:, :, :])
            bias = pp.tile([128, D], F32)
            nc.sync.dma_start(
                out=bias, in_=bass.AP(tensor=bo, offset=0, ap=[[0, 128], [1, D]]))

            catT = pp.tile([128, NG, TQ], BF16, name="catT")
            vt = pp.tile([128, NSC, 16, 65], BF16, name="vt")
            nc.gpsimd.memset(vt[:, :, :, 64:65], 1.0)

            high = deque()   # K/Q proj for the next head group, final proj
            low = deque()    # V-projection chunks (deadline several groups out)
            vlock = {}       # g -> deque of whole V chunks, sc-lockstep drained

            def drain(n):
                for _ in range(n):
                    if high:
                        high.popleft()()
                    elif low:
                        low.popleft()()
                    else:
                        break

            def v_chunk_steps(sc, g0, ng):
                """V proj steps for head groups g0..g0+ng at s-chunk sc."""
                w = ng * 128
                st = {}
                def mm(dc):
                    if dc == 0:
                        st["p"] = psw.tile([128, 512], F32, tag="work",
                                           name=f"vw{g0}_{sc}")
                    nc.tensor.matmul(
                        st["p"][:, 0:w],
                        xT[:, dc, sc * 128:(sc + 1) * 128],
                        wvT[:, dc, g0:g0 + ng, :],
                        start=(dc == 0), stop=(dc == NDC - 1))
                def cp():
                    nc.vector.tensor_copy(
                        out=vt[:, sc, 2 * g0:2 * (g0 + ng), 0:64],
                        in_=st["p"][:, 0:w].rearrange("p (h c) -> p h c", h=2 * ng))
                return [lambda dc=dc: mm(dc) for dc in range(NDC)] + [cp]

            def push_v_low(g0, ng):
                for sc in range(NSC):
                    low.extend(v_chunk_steps(sc, g0, ng))

            def push_v_lock(g, g0, ng):
                dq = deque()
                for sc in range(NSC):
                    def chunk(sc=sc, g0=g0, ng=ng):
                        for f in v_chunk_steps(sc, g0, ng):
                            f()
                    dq.append(chunk)
                vlock[g] = dq

            kqtiles = {}

            def push_proj(g, sbs=(0, 1, 2, 3), qhs=(0, 1)):
                # K/Q projections for head group g -> high queue
                if g in kqtiles:
                    kT, qT = kqtiles[g]
                else:
                    kT = kqp.tile([128, T], BF16, tag="kT")
                    qT = kqp.tile([128, TQ], BF16, tag="qT")
                    kqtiles[g] = (kT, qT)
                for sb in sbs:
                    st = {}
                    def mk(sb=sb, st=st, kT=kT):
                        def mm(dc):
                            if dc == 0:
                                st["p"] = psw.tile([128, 512], F32, tag="work",
                                                   name=f"kw{g}_{sb}")
                            nc.tensor.matmul(
                                st["p"], wkT[:, dc, g, :],
                                xT[:, dc, sb * 512:(sb + 1) * 512],
                                start=(dc == 0), stop=(dc == NDC - 1))
                        def cp():
                            nc.vector.tensor_copy(
                                out=kT[:, sb * 512:(sb + 1) * 512], in_=st["p"])
                        return [lambda dc=dc: mm(dc) for dc in range(NDC)] + [cp]
                    high.extend(mk())
                for qh in qhs:
                    st = {}
                    def mk(qh=qh, st=st, qT=qT):
                        qs = slice(qh * 512, (qh + 1) * 512)
                        def mm(dc):
                            if dc == 0:
                                st["p"] = psw.tile([128, 512], F32, tag="work",
                                                   name=f"qw{g}_{qh}")
                            nc.tensor.matmul(
                                st["p"], wqT[:, dc, g, :],
                                xT[:, dc, qh * 512:(qh + 1) * 512],
                                start=(dc == 0), stop=(dc == NDC - 1))
                        def cp():
                            nc.vector.tensor_copy(out=qT[:, qs], in_=st["p"])
                        return [lambda dc=dc: mm(dc) for dc in range(NDC)] + [cp]
                    high.extend(mk())

            def push_final(qbs):
                # final projection y[qb*128:, :] = catT.T @ woT + bias
                for qb in qbs:
                    yt = yp.tile([128, D], F32, tag="yt", name=f"yt{qb}")
                    for nh in range(2):
                        st = {}
                        def mk(qb=qb, nh=nh, st=st, yt=yt):
                            def mm(g2):
                                if g2 == 0:
                                    st["p"] = psw.tile([128, 512], F32, tag="work",
                                                       name=f"fw{qb}_{nh}")
                                nc.tensor.matmul(
                                    st["p"],
                                    catT[:, g2, qb * 128:(qb + 1) * 128],
                                    woT[:, g2, nh * 512:(nh + 1) * 512],
                                    start=(g2 == 0), stop=(g2 == NG - 1))
                            def add():
                                nc.vector.tensor_add(
                                    out=yt[:, nh * 512:(nh + 1) * 512],
                                    in0=st["p"],
                                    in1=bias[:, nh * 512:(nh + 1) * 512])
                            return ([lambda g2=g2: mm(g2) for g2 in range(NG)]
                                    + [add])
                        high.extend(mk())
                    def out_dma(qb=qb, yt=yt):
                        nc.sync.dma_start(
                            out=y[qb * 128:(qb + 1) * 128, :], in_=yt)
                    high.append(out_dma)

            # V schedule: groups 0,1 lockstep in g0; 2..6 via low queue;
            # group 7 lockstep in g7.
            push_v_lock(0, 0, 2)
            push_proj(0, sbs=(0,), qhs=(0,))
            while high:
                drain(1)
            push_proj(0, sbs=(1, 2, 3), qhs=(1,))
            push_v_low(2, 2)
            push_v_low(4, 2)
            push_v_low(6, 1)
            push_v_lock(7, 7, 1)
            # max low steps allowed to remain at the start of group g
            allowed = {0: 432, 1: 432, 2: 288, 3: 288, 4: 144, 5: 144, 6: 0, 7: 0}

            for g in range(NG):
                if g + 1 < NG:
                    push_proj(g + 1)
                while len(low) > allowed[g]:
                    low.popleft()()
                kT, qT = kqtiles[g]
                vl = vlock.get(g)
                emitted = 0
                for qh in range(2):
                    qs = slice(qh * 512, (qh + 1) * 512)
                    pv0 = psv.tile([65, 512], F32, tag="pv")
                    pv1 = psv.tile([65, 512], F32, tag="pv")
                    for sc in range(NSC):
                        if vl and qh == 0:
                            while vl and emitted < min(sc + 2, NSC):
                                vl.popleft()()
                                emitted += 1
                        lg = psl.tile([128, 2, 512], F32, tag="log")
                        nc.tensor.matmul(
                            lg[:, 0, :], kT[0:64, sc * 128:(sc + 1) * 128],
                            qT[0:64, qs], start=True, stop=True)
                        nc.tensor.matmul(
                            lg[:, 1, :], kT[64:128, sc * 128:(sc + 1) * 128],
                            qT[64:128, qs], start=True, stop=True)
                        pt = ptp.tile([128, 2, 512], BF16, tag="pt")
                        nc.scalar.activation(
                            out=pt.rearrange("p a b -> p (a b)"),
                            in_=lg.rearrange("p a b -> p (a b)"),
                            func=EXP, scale=SCALE)
                        drain(1)
                        nc.tensor.matmul(
                            pv0, vt[:, sc, 2 * g, :], pt[:, 0, :],
                            start=(sc == 0), stop=(sc == NSC - 1))
                        nc.tensor.matmul(
                            pv1, vt[:, sc, 2 * g + 1, :], pt[:, 1, :],
                            start=(sc == 0), stop=(sc == NSC - 1))
                        drain(3)
                    for hloc, pv in ((1, pv1), (0, pv0)):
                        # denominator: reciprocal of PV row 64 on DVE, then
                        # replicate across partitions with a 0-stride DMA
                        rrow = sp.tile([65, 512], F32, tag="rrow")
                        nc.vector.reciprocal(out=rrow[64:65, :], in_=pv[64:65, :])
                        rec = sp.tile([64, 512], F32, tag="rec")
                        row = rrow[64:65, :]
                        brd = bass.AP(
                            tensor=row.tensor, offset=row.offset,
                            ap=[list(row.ap[0]), [0, 64]]
                               + [list(d) for d in row.ap[1:]])
                        nc.sync.dma_start(out=rec, in_=brd)
                        if hloc == 0:
                            nc.vector.tensor_mul(
                                out=catT[0:64, g, qs], in0=pv[0:64, :], in1=rec)
                        else:
                            tmp = sp.tile([64, 512], BF16, tag="tmp")
                            nc.vector.tensor_mul(out=tmp, in0=pv[0:64, :], in1=rec)
                            nc.sync.dma_start(out=catT[64:128, g, qs], in_=tmp)
                    if g == NG - 1:
                        # final projection for the completed query half
                        push_final([0, 1, 2, 3] if qh == 0 else [4, 5, 6, 7])

            while high or low:
                drain(1)

    nc.compile()
    return nc


_CACHE = {}


def _prep_weights(Wq, Wk, Wv, Wo, bo):
    def wt(W):  # [1024 out, 1024 in] -> [128 p, dc, g, 128 j] bf16
        WT = np.ascontiguousarray(W.reshape(D, D)).T  # [in, out]
        return np.ascontiguousarray(
            WT.reshape(NDC, 128, NG, 128).transpose(1, 0, 2, 3)
        ).astype(ml_dtypes.bfloat16)
    wot = np.ascontiguousarray(
        Wo.T.reshape(NG, 128, D).transpose(1, 0, 2)).astype(ml_dtypes.bfloat16)
    bo2 = np.ascontiguousarray(bo.reshape(1, D)).astype(np.float32)
    return wt(Wq), wt(Wk), wt(Wv), wot, bo2


def kernel(x, Wq, Wk, Wv, Wo, bo):
    if "nc" not in _CACHE:
        _CACHE["nc"] = build_nc()
    nc = _CACHE["nc"]
    wq2, wk2, wv2, wot, bo2 = _prep_weights(
        np.asarray(Wq, np.float32), np.asarray(Wk, np.float32),
        np.asarray(Wv, np.float32), np.asarray(Wo, np.float32),
        np.asarray(bo, np.float32))
    x = np.asarray(x, np.float32)
    in_maps = []
    for c in range(8):
        b, h = c // 2, c % 2
        xb = x[b] if h == 0 else np.concatenate([x[b, TQ:], x[b, :TQ]], axis=0)
        # x^T [1024 d, 2048 s] -> [128 p, dc, s] bf16
        xT = np.ascontiguousarray(
            xb.T.reshape(NDC, 128, T).transpose(1, 0, 2)).astype(ml_dtypes.bfloat16)
        in_maps.append({"xin": xT, "wqt": wq2, "wkt": wk2, "wvt": wv2,
                        "wot": wot, "bo": bo2,
                        "wk01": np.ascontiguousarray(wk2[:, :, 0:2, :]),
                        "wq01": np.ascontiguousarray(wq2[:, :, 0:2, :]),
                        "wv01": np.ascontiguousarray(wv2[:, :, 0:2, :])})
    res = run_bass_kernel_spmd(nc, in_maps, core_ids=list(range(8)))
    out = np.empty((4, T, D), dtype=np.float32)
    for c in range(8):
        b, h = c // 2, c % 2
        out[b, h * TQ:(h + 1) * TQ] = res.results[c]["y"]
    return out


# revision 47
# speedup vs baseline: 1.0438x; 1.0113x over previous
"""MHA kernel for TRN2: B=4,T=2048,D=1024,H=16,HD=64 across 8 NeuronCores.

Sharding: core c -> batch c//2, query half c%2 (host rotates the sequence so
each core's queries are columns 0:1024; softmax over keys is permutation
invariant). No collectives.

v5.5 (422us sim vs 600us baseline): all-bf16 matmul operands (rel err
~3e-3), host-side pre-transpose + cast
(no PE transposes), software-pipelined emission: K/Q projections for the next
head group, V-projection chunks, and the final projection are split into
single-matmul steps and drained from work queues inside the attention loop, so
the in-order PE stream always has independent work while PV waits on ACT exp.
V chunks for group 0 / group 7 are emitted in sc-lockstep inside that group's
own attention (3-chunk lookahead). Softmax denominators come from a ones
column in V ([65,512] PV psum row 64), replicated across partitions with a
0-stride-AP SBUF DMA broadcast, then DVE reciprocal+multiply. Dedicated small
DRAM tensors carry the group-0/1 weight slices so the first attention group
starts after ~2.5MB of DMA instead of 7MB.
"""
import sys
sys.path.insert(0, "/opt/trn_rl_repo")
import warnings
warnings.filterwarnings("ignore")

from collections import deque

import numpy as np
import ml_dtypes
import concourse.bass as bass
import concourse.mybir as mybir
import concourse.tile as tile
from concourse import bacc
from concourse.bass_utils import run_bass_kernel_spmd

F32 = mybir.dt.float32
BF16 = mybir.dt.bfloat16
EXP = mybir.ActivationFunctionType.Exp

T, D = 2048, 1024
TQ = 1024          # queries per core
NG = 8             # head groups (2 heads each)
NSC = 16           # s chunks of 128
NDC = 8            # d chunks of 128
SCALE = 0.125      # 1/sqrt(64)


def build_nc():
    nc = bacc.Bacc("TRN2", target_bir_lowering=False, debug=False, num_devices=8)
    # host-prepped layouts (partition-major, bf16)
    xin = nc.dram_tensor("xin", [128, NDC, T], BF16, kind="ExternalInput")
    wqt = nc.dram_tensor("wqt", [128, NDC, NG, 128], BF16, kind="ExternalInput")
    wkt = nc.dram_tensor("wkt", [128, NDC, NG, 128], BF16, kind="ExternalInput")
    wvt = nc.dram_tensor("wvt", [128, NDC, NG, 128], BF16, kind="ExternalInput")
    wot = nc.dram_tensor("wot", [128, NG, D], BF16, kind="ExternalInput")
    wk01 = nc.dram_tensor("wk01", [128, NDC, 2, 128], BF16, kind="ExternalInput")
    wq01 = nc.dram_tensor("wq01", [128, NDC, 2, 128], BF16, kind="ExternalInput")
    wv01 = nc.dram_tensor("wv01", [128, NDC, 2, 128], BF16, kind="ExternalInput")
    bo = nc.dram_tensor("bo", [1, D], F32, kind="ExternalInput")
    y = nc.dram_tensor("y", [TQ, D], F32, kind="ExternalOutput")

    with tile.TileContext(nc) as tc:
        with (
            tc.tile_pool(name="persist", bufs=1) as pp,
            tc.tile_pool(name="kq", bufs=2) as kqp,
            tc.tile_pool(name="ptp", bufs=5) as ptp,
            tc.tile_pool(name="small", bufs=4) as sp,
            tc.tile_pool(name="yt", bufs=3) as yp,
            tc.tile_pool(name="ps_work", bufs=2, space="PSUM") as psw,
            tc.tile_pool(name="ps_log", bufs=2, space="PSUM") as psl,
            tc.tile_pool(name="ps_pv", bufs=2, space="PSUM") as psv,
        ):
            # input DMAs: interleave weight and x chunks so the in-order PE
            # stream unblocks as early as possible
            wkT = pp.tile([128, NDC, NG, 128], BF16, name="wkT")
            wqT = pp.tile([128, NDC, NG, 128], BF16, name="wqT")
            wvT = pp.tile([128, NDC, NG, 128], BF16, name="wvT")
            xT = pp.tile([128, NDC, T], BF16, name="xT")
            nc.sync.dma_start(out=wkT[:, :, 0:2, :], in_=wk01[:, :, :, :])
            for dc in range(NDC):
                nc.sync.dma_start(out=xT[:, dc, 0:512], in_=xin[:, dc, 0:512])
            nc.sync.dma_start(out=wqT[:, :, 0:2, :], in_=wq01[:, :, :, :])
            nc.sync.dma_start(out=wvT[:, :, 0:2, :], in_=wv01[:, :, :, :])
            for dc in range(NDC):
                nc.sync.dma_start(out=xT[:, dc, 512:1024], in_=xin[:, dc, 512:1024])
            nc.sync.dma_start(out=wkT[:, :, 2:8, :], in_=wkt[:, :, 2:8, :])
            nc.sync.dma_start(out=wqT[:, :, 2:8, :], in_=wqt[:, :, 2:8, :])
            for dc in range(NDC):
                nc.sync.dma_start(out=xT[:, dc, 1024:1536], in_=xin[:, dc, 1024:1536])
            nc.sync.dma_start(out=wvT[:, :, 2:8, :], in_=wvt[:, :, 2:8, :])
            for dc in range(NDC):
                nc.sync.dma_start(out=xT[:, dc, 1536:2048], in_=xin[:, dc, 1536:2048])
            woT = pp.tile([128, NG, D], BF16, name="woT")
            nc.sync.dma_start(out=woT, in_=wot[:, :, :])
            bias = pp.tile([128, D], F32)
            nc.sync.dma_start(
                out=bias, in_=bass.AP(tensor=bo, offset=0, ap=[[0, 128], [1, D]]))

            catT = pp.tile([128, NG, TQ], BF16, name="catT")
            vt = pp.tile([128, NSC, 16, 65], BF16, name="vt")
            nc.gpsimd.memset(vt[:, :, :, 64:65], 1.0)

            high = deque()   # K/Q proj for the next head group, final proj
            low = deque()    # V-projection chunks (deadline several groups out)
            vlock = {}       # g -> deque of whole V chunks, sc-lockstep drained

            def drain(n):
                for _ in range(n):
                    if high:
                        high.popleft()()
                    elif low:
                        low.popleft()()
                    else:
                        break

            def v_chunk_steps(sc, g0, ng):
                """V proj steps for head groups g0..g0+ng at s-chunk sc."""
                w = ng * 128
                st = {}
                def mm(dc):
                    if dc == 0:
                        st["p"] = psw.tile([128, 512], F32, tag="work",
                                           name=f"vw{g0}_{sc}")
                    nc.tensor.matmul(
                        st["p"][:, 0:w],
                        xT[:, dc, sc * 128:(sc + 1) * 128],
                        wvT[:, dc, g0:g0 + ng, :],
                        start=(dc == 0), stop=(dc == NDC - 1))
                def cp():
                    nc.vector.tensor_copy(
                        out=vt[:, sc, 2 * g0:2 * (g0 + ng), 0:64],
                        in_=st["p"][:, 0:w].rearrange("p (h c) -> p h c", h=2 * ng))
                return [lambda dc=dc: mm(dc) for dc in range(NDC)] + [cp]

            def push_v_low(g0, ng):
                for sc in range(NSC):
                    low.extend(v_chunk_steps(sc, g0, ng))

            def push_v_lock(g, g0, ng):
                dq = deque()
                for sc in range(NSC):
                    def chunk(sc=sc, g0=g0, ng=ng):
                        for f in v_chunk_steps(sc, g0, ng):
                            f()
                    dq.append(chunk)
                vlock[g] = dq

            kqtiles = {}

            def push_proj(g, sbs=(0, 1, 2, 3), qhs=(0, 1)):
                # K/Q projections for head group g -> high queue
                if g in kqtiles:
                    kT, qT = kqtiles[g]
                else:
                    kT = kqp.tile([128, T], BF16, tag="kT")
                    qT = kqp.tile([128, TQ], BF16, tag="qT")
                    kqtiles[g] = (kT, qT)
                for sb in sbs:
                    st = {}
                    def mk(sb=sb, st=st, kT=kT):
                        def mm(dc):
                            if dc == 0:
                                st["p"] = psw.tile([128, 512], F32, tag="work",
                                                   name=f"kw{g}_{sb}")
                            nc.tensor.matmul(
                                st["p"], wkT[:, dc, g, :],
                                xT[:, dc, sb * 512:(sb + 1) * 512],
                                start=(dc == 0), stop=(dc == NDC - 1))
                        def cp():
                            nc.vector.tensor_copy(
                                out=kT[:, sb * 512:(sb + 1) * 512], in_=st["p"])
                        return [lambda dc=dc: mm(dc) for dc in range(NDC)] + [cp]
                    high.extend(mk())
                for qh in qhs:
                    st = {}
                    def mk(qh=qh, st=st, qT=qT):
                        qs = slice(qh * 512, (qh + 1) * 512)
                        def mm(dc):
                            if dc == 0:
                                st["p"] = psw.tile([128, 512], F32, tag="work",
                                                   name=f"qw{g}_{qh}")
                            nc.tensor.matmul(
                                st["p"], wqT[:, dc, g, :],
                                xT[:, dc, qh * 512:(qh + 1) * 512],
                                start=(dc == 0), stop=(dc == NDC - 1))
                        def cp():
                            nc.vector.tensor_copy(out=qT[:, qs], in_=st["p"])
                        return [lambda dc=dc: mm(dc) for dc in range(NDC)] + [cp]
                    high.extend(mk())

            def push_final(qbs):
                # final projection y[qb*128:, :] = catT.T @ woT + bias
                for qb in qbs:
                    yt = yp.tile([128, D], F32, tag="yt", name=f"yt{qb}")
                    for nh in range(2):
                        st = {}
                        def mk(qb=qb, nh=nh, st=st, yt=yt):
                            def mm(g2):
                                if g2 == 0:
                                    st["p"] = psw.tile([128, 512], F32, tag="work",
                                                       name=f"fw{qb}_{nh}")
                                nc.tensor.matmul(
                                    st["p"],
                                    catT[:, g2, qb * 128:(qb + 1) * 128],
                                    woT[:, g2, nh * 512:(nh + 1) * 512],
                                    start=(g2 == 0), stop=(g2 == NG - 1))
                            def add():
                                nc.vector.tensor_add(
                                    out=yt[:, nh * 512:(nh + 1) * 512],
                                    in0=st["p"],
                                    in1=bias[:, nh * 512:(nh + 1) * 512])
                            def out_dma():
                                nc.sync.dma_start(
                                    out=y[qb * 128:(qb + 1) * 128,
                                          nh * 512:(nh + 1) * 512],
                                    in_=yt[:, nh * 512:(nh + 1) * 512])
                            return ([lambda g2=g2: mm(g2) for g2 in range(NG)]
                                    + [add, out_dma])
                        high.extend(mk())

            # V schedule: groups 0,1 lockstep in g0; 2..6 via low queue;
            # group 7 lockstep in g7.
            push_v_lock(0, 0, 2)
            push_proj(0, sbs=(0,), qhs=(0,))
            while high:
                drain(1)
            push_proj(0, sbs=(1, 2, 3), qhs=(1,))
            push_v_low(2, 2)
            push_v_low(4, 2)
            push_v_low(6, 1)
            push_v_lock(7, 7, 1)
            # max low steps allowed to remain at the start of group g
            allowed = {0: 432, 1: 432, 2: 288, 3: 288, 4: 144, 5: 144, 6: 0, 7: 0}

            for g in range(NG):
                if g + 1 < NG:
                    push_proj(g + 1)
                while len(low) > allowed[g]:
                    low.popleft()()
                kT, qT = kqtiles[g]
                vl = vlock.get(g)
                emitted = 0
                for qh in range(2):
                    qs = slice(qh * 512, (qh + 1) * 512)
                    pv0 = psv.tile([65, 512], F32, tag="pv")
                    pv1 = psv.tile([65, 512], F32, tag="pv")
                    for sc in range(NSC):
                        if vl and qh == 0:
                            while vl and emitted < min(sc + 2, NSC):
                                vl.popleft()()
                                emitted += 1
                        lg = psl.tile([128, 2, 512], F32, tag="log")
                        nc.tensor.matmul(
                            lg[:, 0, :], kT[0:64, sc * 128:(sc + 1) * 128],
                            qT[0:64, qs], start=True, stop=True)
                        nc.tensor.matmul(
                            lg[:, 1, :], kT[64:128, sc * 128:(sc + 1) * 128],
                            qT[64:128, qs], start=True, stop=True)
                        pt = ptp.tile([128, 2, 512], BF16, tag="pt")
                        nc.scalar.activation(
                            out=pt.rearrange("p a b -> p (a b)"),
                            in_=lg.rearrange("p a b -> p (a b)"),
                            func=EXP, scale=SCALE)
                        drain(1)
                        nc.tensor.matmul(
                            pv0, vt[:, sc, 2 * g, :], pt[:, 0, :],
                            start=(sc == 0), stop=(sc == NSC - 1))
                        nc.tensor.matmul(
                            pv1, vt[:, sc, 2 * g + 1, :], pt[:, 1, :],
                            start=(sc == 0), stop=(sc == NSC - 1))
                        drain(3)
                    for hloc, pv in ((1, pv1), (0, pv0)):
                        # denominator: reciprocal of PV row 64 on DVE, then
                        # replicate across partitions with a 0-stride DMA
                        rrow = sp.tile([65, 512], F32, tag="rrow")
                        nc.vector.reciprocal(out=rrow[64:65, :], in_=pv[64:65, :])
                        rec = sp.tile([64, 512], F32, tag="rec")
                        row = rrow[64:65, :]
                        brd = bass.AP(
                            tensor=row.tensor, offset=row.offset,
                            ap=[list(row.ap[0]), [0, 64]]
                               + [list(d) for d in row.ap[1:]])
                        nc.sync.dma_start(out=rec, in_=brd)
                        if hloc == 0:
                            nc.vector.tensor_mul(
                                out=catT[0:64, g, qs], in0=pv[0:64, :], in1=rec)
                        else:
                            tmp = sp.tile([64, 512], BF16, tag="tmp")
                            nc.vector.tensor_mul(out=tmp, in0=pv[0:64, :], in1=rec)
                            nc.sync.dma_start(out=catT[64:128, g, qs], in_=tmp)
                    if g == NG - 1:
                        # final projection for the completed query half
                        push_final([0, 1, 2, 3] if qh == 0 else [4, 5, 6, 7])

            while high or low:
                drain(1)

    nc.compile()
    return nc


_CACHE = {}


def _prep_weights(Wq, Wk, Wv, Wo, bo):
    def wt(W):  # [1024 out, 1024 in] -> [128 p, dc, g, 128 j] bf16
        WT = np.ascontiguousarray(W.reshape(D, D)).T  # [in, out]
        return np.ascontiguousarray(
            WT.reshape(NDC, 128, NG, 128).transpose(1, 0, 2, 3)
        ).astype(ml_dtypes.bfloat16)
    wot = np.ascontiguousarray(
        Wo.T.reshape(NG, 128, D).transpose(1, 0, 2)).astype(ml_dtypes.bfloat16)
    bo2 = np.ascontiguousarray(bo.reshape(1, D)).astype(np.float32)
    return wt(Wq), wt(Wk), wt(Wv), wot, bo2


def kernel(x, Wq, Wk, Wv, Wo, bo):
    if "nc" not in _CACHE:
        _CACHE["nc"] = build_nc()
    nc = _CACHE["nc"]
    wq2, wk2, wv2, wot, bo2 = _prep_weights(
        np.asarray(Wq, np.float32), np.asarray(Wk, np.float32),
        np.asarray(Wv, np.float32), np.asarray(Wo, np.float32),
        np.asarray(bo, np.float32))
    x = np.asarray(x, np.float32)
    in_maps = []
    for c in range(8):
        b, h = c // 2, c % 2
        xb = x[b] if h == 0 else np.concatenate([x[b, TQ:], x[b, :TQ]], axis=0)
        # x^T [1024 d, 2048 s] -> [128 p, dc, s] bf16
        xT = np.ascontiguousarray(
            xb.T.reshape(NDC, 128, T).transpose(1, 0, 2)).astype(ml_dtypes.bfloat16)
        in_maps.append({"xin": xT, "wqt": wq2, "wkt": wk2, "wvt": wv2,
                        "wot": wot, "bo": bo2,
                        "wk01": np.ascontiguousarray(wk2[:, :, 0:2, :]),
                        "wq01": np.ascontiguousarray(wq2[:, :, 0:2, :]),
                        "wv01": np.ascontiguousarray(wv2[:, :, 0:2, :])})
    res = run_bass_kernel_spmd(nc, in_maps, core_ids=list(range(8)))
    out = np.empty((4, T, D), dtype=np.float32)
    for c in range(8):
        b, h = c // 2, c % 2
        out[b, h * TQ:(h + 1) * TQ] = res.results[c]["y"]
    return out


# revision 49
# speedup vs baseline: 1.0785x; 1.0333x over previous
"""MHA kernel for TRN2: B=4,T=2048,D=1024,H=16,HD=64 across 8 NeuronCores.

Sharding: core c -> batch c//2, query half c%2 (host rotates the sequence so
each core's queries are columns 0:1024; softmax over keys is permutation
invariant). No collectives.

v5.5 (422us sim vs 600us baseline): all-bf16 matmul operands (rel err
~3e-3), host-side pre-transpose + cast
(no PE transposes), software-pipelined emission: K/Q projections for the next
head group, V-projection chunks, and the final projection are split into
single-matmul steps and drained from work queues inside the attention loop, so
the in-order PE stream always has independent work while PV waits on ACT exp.
V chunks for group 0 / group 7 are emitted in sc-lockstep inside that group's
own attention (3-chunk lookahead). Softmax denominators come from a ones
column in V ([65,512] PV psum row 64), replicated across partitions with a
0-stride-AP SBUF DMA broadcast, then DVE reciprocal+multiply. Dedicated small
DRAM tensors carry the group-0/1 weight slices so the first attention group
starts after ~2.5MB of DMA instead of 7MB.
"""
import sys
sys.path.insert(0, "/opt/trn_rl_repo")
import warnings
warnings.filterwarnings("ignore")

from collections import deque

import numpy as np
import ml_dtypes
import concourse.bass as bass
import concourse.mybir as mybir
import concourse.tile as tile
from concourse import bacc
from concourse.bass_utils import run_bass_kernel_spmd

F32 = mybir.dt.float32
BF16 = mybir.dt.bfloat16
EXP = mybir.ActivationFunctionType.Exp

T, D = 2048, 1024
TQ = 1024          # queries per core
NG = 8             # head groups (2 heads each)
NSC = 16           # s chunks of 128
NDC = 8            # d chunks of 128
SCALE = 0.125      # 1/sqrt(64)


def build_nc():
    nc = bacc.Bacc("TRN2", target_bir_lowering=False, debug=False, num_devices=8)
    # host-prepped layouts (partition-major, bf16)
    xin = nc.dram_tensor("xin", [128, NDC, T], BF16, kind="ExternalInput")
    wqt = nc.dram_tensor("wqt", [128, NDC, NG, 128], BF16, kind="ExternalInput")
    wkt = nc.dram_tensor("wkt", [128, NDC, NG, 128], BF16, kind="ExternalInput")
    wvt = nc.dram_tensor("wvt", [128, NDC, NG, 128], BF16, kind="ExternalInput")
    wot = nc.dram_tensor("wot", [128, NG, D], BF16, kind="ExternalInput")
    wk01 = nc.dram_tensor("wk01", [128, NDC, 2, 128], BF16, kind="ExternalInput")
    wq01 = nc.dram_tensor("wq01", [128, NDC, 2, 128], BF16, kind="ExternalInput")
    wv01 = nc.dram_tensor("wv01", [128, NDC, 2, 128], BF16, kind="ExternalInput")
    bo = nc.dram_tensor("bo", [1, D], F32, kind="ExternalInput")
    y = nc.dram_tensor("y", [TQ, D], F32, kind="ExternalOutput")

    with tile.TileContext(nc) as tc:
        with (
            tc.tile_pool(name="persist", bufs=1) as pp,
            tc.tile_pool(name="kq", bufs=2) as kqp,
            tc.tile_pool(name="ptp", bufs=6) as ptp,
            tc.tile_pool(name="small", bufs=4) as sp,
            tc.tile_pool(name="yt", bufs=3) as yp,
            tc.tile_pool(name="ps_work", bufs=2, space="PSUM") as psw,
            tc.tile_pool(name="ps_log", bufs=2, space="PSUM") as psl,
            tc.tile_pool(name="ps_pv", bufs=2, space="PSUM") as psv,
        ):
            # input DMAs: interleave weight and x chunks so the in-order PE
            # stream unblocks as early as possible
            wkT = pp.tile([128, NDC, NG, 128], BF16, name="wkT")
            wqT = pp.tile([128, NDC, NG, 128], BF16, name="wqT")
            wvT = pp.tile([128, NDC, NG, 128], BF16, name="wvT")
            xT = pp.tile([128, NDC, T], BF16, name="xT")
            nc.sync.dma_start(out=wkT[:, :, 0:2, :], in_=wk01[:, :, :, :])
            for dc in range(NDC):
                nc.sync.dma_start(out=xT[:, dc, 0:512], in_=xin[:, dc, 0:512])
            nc.sync.dma_start(out=wqT[:, :, 0:2, :], in_=wq01[:, :, :, :])
            nc.sync.dma_start(out=wvT[:, :, 0:2, :], in_=wv01[:, :, :, :])
            for dc in range(NDC):
                nc.sync.dma_start(out=xT[:, dc, 512:1024], in_=xin[:, dc, 512:1024])
            nc.sync.dma_start(out=wkT[:, :, 2:8, :], in_=wkt[:, :, 2:8, :])
            nc.sync.dma_start(out=wqT[:, :, 2:8, :], in_=wqt[:, :, 2:8, :])
            for dc in range(NDC):
                nc.sync.dma_start(out=xT[:, dc, 1024:1536], in_=xin[:, dc, 1024:1536])
            nc.sync.dma_start(out=wvT[:, :, 2:8, :], in_=wvt[:, :, 2:8, :])
            for dc in range(NDC):
                nc.sync.dma_start(out=xT[:, dc, 1536:2048], in_=xin[:, dc, 1536:2048])
            woT = pp.tile([128, NG, D], BF16, name="woT")
            nc.sync.dma_start(out=woT, in_=wot[:, :, :])
            bias = pp.tile([128, D], F32)
            nc.sync.dma_start(
                out=bias, in_=bass.AP(tensor=bo, offset=0, ap=[[0, 128], [1, D]]))

            catT = pp.tile([128, NG, TQ], BF16, name="catT")
            vt = pp.tile([128, NSC, 16, 65], BF16, name="vt")
            nc.gpsimd.memset(vt[:, :, :, 64:65], 1.0)

            high = deque()   # K/Q proj for the next head group, final proj
            low = deque()    # V-projection chunks (deadline several groups out)
            vlock = {}       # g -> deque of whole V chunks, sc-lockstep drained

            def drain(n):
                for _ in range(n):
                    if high:
                        high.popleft()()
                    elif low:
                        low.popleft()()
                    else:
                        break

            def v_chunk_steps(sc, g0, ng):
                """V proj steps for head groups g0..g0+ng at s-chunk sc."""
                w = ng * 128
                st = {}
                def mm(dc):
                    if dc == 0:
                        st["p"] = psw.tile([128, 512], F32, tag="work",
                                           name=f"vw{g0}_{sc}")
                    nc.tensor.matmul(
                        st["p"][:, 0:w],
                        xT[:, dc, sc * 128:(sc + 1) * 128],
                        wvT[:, dc, g0:g0 + ng, :],
                        start=(dc == 0), stop=(dc == NDC - 1))
                def cp():
                    nc.vector.tensor_copy(
                        out=vt[:, sc, 2 * g0:2 * (g0 + ng), 0:64],
                        in_=st["p"][:, 0:w].rearrange("p (h c) -> p h c", h=2 * ng))
                return [lambda dc=dc: mm(dc) for dc in range(NDC)] + [cp]

            def push_v_low(g0, ng):
                for sc in range(NSC):
                    low.extend(v_chunk_steps(sc, g0, ng))

            def push_v_lock(g, g0, ng):
                dq = deque()
                for sc in range(NSC):
                    def chunk(sc=sc, g0=g0, ng=ng):
                        for f in v_chunk_steps(sc, g0, ng):
                            f()
                    dq.append(chunk)
                vlock[g] = dq

            kqtiles = {}

            def push_proj(g, sbs=(0, 1, 2, 3), qhs=(0, 1)):
                # K/Q projections for head group g -> high queue
                if g in kqtiles:
                    kT, qT = kqtiles[g]
                else:
                    kT = kqp.tile([128, T], BF16, tag="kT")
                    qT = kqp.tile([128, TQ], BF16, tag="qT")
                    kqtiles[g] = (kT, qT)
                for sb in sbs:
                    st = {}
                    def mk(sb=sb, st=st, kT=kT):
                        def mm(dc):
                            if dc == 0:
                                st["p"] = psw.tile([128, 512], F32, tag="work",
                                                   name=f"kw{g}_{sb}")
                            nc.tensor.matmul(
                                st["p"], wkT[:, dc, g, :],
                                xT[:, dc, sb * 512:(sb + 1) * 512],
                                start=(dc == 0), stop=(dc == NDC - 1))
                        def cp():
                            nc.vector.tensor_copy(
                                out=kT[:, sb * 512:(sb + 1) * 512], in_=st["p"])
                        return [lambda dc=dc: mm(dc) for dc in range(NDC)] + [cp]
                    high.extend(mk())
                for qh in qhs:
                    st = {}
                    def mk(qh=qh, st=st, qT=qT):
                        qs = slice(qh * 512, (qh + 1) * 512)
                        def mm(dc):
                            if dc == 0:
                                st["p"] = psw.tile([128, 512], F32, tag="work",
                                                   name=f"qw{g}_{qh}")
                            nc.tensor.matmul(
                                st["p"], wqT[:, dc, g, :],
                                xT[:, dc, qh * 512:(qh + 1) * 512],
                                start=(dc == 0), stop=(dc == NDC - 1))
                        def cp():
                            nc.vector.tensor_copy(out=qT[:, qs], in_=st["p"])
                        return [lambda dc=dc: mm(dc) for dc in range(NDC)] + [cp]
                    high.extend(mk())

            def push_final(qbs):
                # final projection y[qb*128:, :] = catT.T @ woT + bias
                for qb in qbs:
                    yt = yp.tile([128, D], F32, tag="yt", name=f"yt{qb}")
                    for nh in range(2):
                        st = {}
                        def mk(qb=qb, nh=nh, st=st, yt=yt):
                            def mm(g2):
                                if g2 == 0:
                                    st["p"] = psw.tile([128, 512], F32, tag="work",
                                                       name=f"fw{qb}_{nh}")
                                nc.tensor.matmul(
                                    st["p"],
                                    catT[:, g2, qb * 128:(qb + 1) * 128],
                                    woT[:, g2, nh * 512:(nh + 1) * 512],
                                    start=(g2 == 0), stop=(g2 == NG - 1))
                            def add():
                                nc.vector.tensor_add(
                                    out=yt[:, nh * 512:(nh + 1) * 512],
                                    in0=st["p"],
                                    in1=bias[:, nh * 512:(nh + 1) * 512])
                            def out_dma():
                                nc.sync.dma_start(
                                    out=y[qb * 128:(qb + 1) * 128,
                                          nh * 512:(nh + 1) * 512],
                                    in_=yt[:, nh * 512:(nh + 1) * 512])
                            return ([lambda g2=g2: mm(g2) for g2 in range(NG)]
                                    + [add, out_dma])
                        high.extend(mk())

            # V schedule: groups 0,1 lockstep in g0; 2..6 via low queue;
            # group 7 lockstep in g7.
            push_v_lock(0, 0, 2)
            push_proj(0, sbs=(0,), qhs=(0,))
            while high:
                drain(1)
            push_proj(0, sbs=(1, 2, 3), qhs=(1,))
            push_v_low(2, 2)
            push_v_low(4, 2)
            push_v_low(6, 1)
            push_v_lock(7, 7, 1)
            # max low steps allowed to remain at the start of group g
            allowed = {0: 432, 1: 432, 2: 288, 3: 288, 4: 144, 5: 144, 6: 0, 7: 0}

            for g in range(NG):
                if g + 1 < NG:
                    push_proj(g + 1)
                while len(low) > allowed[g]:
                    low.popleft()()
                kT, qT = kqtiles[g]
                vl = vlock.get(g)
                emitted = 0
                for qh in range(2):
                    qs = slice(qh * 512, (qh + 1) * 512)
                    pv0 = psv.tile([65, 512], F32, tag="pv")
                    pv1 = psv.tile([65, 512], F32, tag="pv")
                    pt_prev = None
                    for sc in range(NSC):
                        if vl and qh == 0:
                            while vl and emitted < min(sc + 2, NSC):
                                vl.popleft()()
                                emitted += 1
                        lg = psl.tile([128, 2, 512], F32, tag="log")
                        nc.tensor.matmul(
                            lg[:, 0, :], kT[0:64, sc * 128:(sc + 1) * 128],
                            qT[0:64, qs], start=True, stop=True)
                        nc.tensor.matmul(
                            lg[:, 1, :], kT[64:128, sc * 128:(sc + 1) * 128],
                            qT[64:128, qs], start=True, stop=True)
                        pt = ptp.tile([128, 2, 512], BF16, tag="pt")
                        nc.scalar.activation(
                            out=pt.rearrange("p a b -> p (a b)"),
                            in_=lg.rearrange("p a b -> p (a b)"),
                            func=EXP, scale=SCALE)
                        drain(1)
                        if pt_prev is not None:
                            nc.tensor.matmul(
                                pv0, vt[:, sc - 1, 2 * g, :], pt_prev[:, 0, :],
                                start=(sc == 1), stop=False)
                            nc.tensor.matmul(
                                pv1, vt[:, sc - 1, 2 * g + 1, :],
                                pt_prev[:, 1, :],
                                start=(sc == 1), stop=False)
                        pt_prev = pt
                        drain(3)
                    nc.tensor.matmul(
                        pv0, vt[:, NSC - 1, 2 * g, :], pt_prev[:, 0, :],
                        start=False, stop=True)
                    nc.tensor.matmul(
                        pv1, vt[:, NSC - 1, 2 * g + 1, :], pt_prev[:, 1, :],
                        start=False, stop=True)
                    for hloc, pv in ((1, pv1), (0, pv0)):
                        # denominator: reciprocal of PV row 64 on DVE, then
                        # replicate across partitions with a 0-stride DMA
                        rrow = sp.tile([65, 512], F32, tag="rrow")
                        nc.vector.reciprocal(out=rrow[64:65, :], in_=pv[64:65, :])
                        rec = sp.tile([64, 512], F32, tag="rec")
                        row = rrow[64:65, :]
                        brd = bass.AP(
                            tensor=row.tensor, offset=row.offset,
                            ap=[list(row.ap[0]), [0, 64]]
                               + [list(d) for d in row.ap[1:]])
                        nc.sync.dma_start(out=rec, in_=brd)
                        if hloc == 0:
                            nc.vector.tensor_mul(
                                out=catT[0:64, g, qs], in0=pv[0:64, :], in1=rec)
                        else:
                            tmp = sp.tile([64, 512], BF16, tag="tmp")
                            nc.vector.tensor_mul(out=tmp, in0=pv[0:64, :], in1=rec)
                            nc.sync.dma_start(out=catT[64:128, g, qs], in_=tmp)
                    if g == NG - 1:
                        # final projection for the completed query half
                        push_final([0, 1, 2, 3] if qh == 0 else [4, 5, 6, 7])

            while high or low:
                drain(1)

    nc.compile()
    return nc


_CACHE = {}


def _prep_weights(Wq, Wk, Wv, Wo, bo):
    def wt(W):  # [1024 out, 1024 in] -> [128 p, dc, g, 128 j] bf16
        WT = np.ascontiguousarray(W.reshape(D, D)).T  # [in, out]
        return np.ascontiguousarray(
            WT.reshape(NDC, 128, NG, 128).transpose(1, 0, 2, 3)
        ).astype(ml_dtypes.bfloat16)
    wot = np.ascontiguousarray(
        Wo.T.reshape(NG, 128, D).transpose(1, 0, 2)).astype(ml_dtypes.bfloat16)
    bo2 = np.ascontiguousarray(bo.reshape(1, D)).astype(np.float32)
    return wt(Wq), wt(Wk), wt(Wv), wot, bo2


def kernel(x, Wq, Wk, Wv, Wo, bo):
    if "nc" not in _CACHE:
        _CACHE["nc"] = build_nc()
    nc = _CACHE["nc"]
    wq2, wk2, wv2, wot, bo2 = _prep_weights(
        np.asarray(Wq, np.float32), np.asarray(Wk, np.float32),
        np.asarray(Wv, np.float32), np.asarray(Wo, np.float32),
        np.asarray(bo, np.float32))
    x = np.asarray(x, np.float32)
    in_maps = []
    for c in range(8):
        b, h = c // 2, c % 2
        xb = x[b] if h == 0 else np.concatenate([x[b, TQ:], x[b, :TQ]], axis=0)
        # x^T [1024 d, 2048 s] -> [128 p, dc, s] bf16
        xT = np.ascontiguousarray(
            xb.T.reshape(NDC, 128, T).transpose(1, 0, 2)).astype(ml_dtypes.bfloat16)
        in_maps.append({"xin": xT, "wqt": wq2, "wkt": wk2, "wvt": wv2,
                        "wot": wot, "bo": bo2,
                        "wk01": np.ascontiguousarray(wk2[:, :, 0:2, :]),
                        "wq01": np.ascontiguousarray(wq2[:, :, 0:2, :]),
                        "wv01": np.ascontiguousarray(wv2[:, :, 0:2, :])})
    res = run_bass_kernel_spmd(nc, in_maps, core_ids=list(range(8)))
    out = np.empty((4, T, D), dtype=np.float32)
    for c in range(8):
        b, h = c // 2, c % 2
        out[b, h * TQ:(h + 1) * TQ] = res.results[c]["y"]
    return out
